# revision 7
# baseline (speedup 1.0000x reference)
"""Trainium2 Bass kernel for a pre-norm transformer decoder layer.

Model: x = x + Attn(LN1(x));  x = x + FFN(LN2(x))
Shapes: x [2, 2048, 1024], H=16 heads, DK=64, FF=4096, f32 I/O.

Sharding over 8 NeuronCores, no collectives:
  core c -> batch entry b = c//4, query rows q0 = (c%4)*512 .. +512.
  Each core computes K/V for its full batch entry (work duplicated 4x
  within the batch group -- cheaper than an on-chip all-gather here),
  and Q/attention/FFN only for its own 512 tokens. The host slices
  inputs per core and concatenates the 8 [512,1024] outputs.

Compute: bf16 matmuls with f32 PSUM accumulation; LN stats, softmax
and residuals in f32. Attention uses S^T-layout scores (k on
partitions) so softmax-exp output E^T feeds the PV matmul directly as
the stationary operand, with a ones-column appended to V to produce
the softmax denominator in the same accumulation (no transposes of P).
All bias additions are folded into the PE as rank-1 outer products.
"""

import numpy as np

import concourse.bass as bass
import concourse.mybir as mybir
import concourse.tile as tile
from concourse.masks import make_identity
from concourse.vector_clock import ScopedClock, VectorClock

F32 = mybir.dt.float32
BF16 = mybir.dt.bfloat16
AF = mybir.ActivationFunctionType
OP = mybir.AluOpType
P = 128


def _bcast(v, p):
    """[D] AP -> [p, D] AP replicated over partitions (step-0 partition dim)."""
    return bass.AP(tensor=v.tensor, offset=v.offset, ap=[[0, p], *v.ap])


class _TC(tile.TileContext):
    """TileContext whose tail drain splits its semaphore waits across
    single-wait NOPs -- this walrus build rejects several sync waits on
    one CTRL instruction ("Too many sync wait commands")."""

    def _drain_and_barrier(self, tick_clock, wait_clock):
        gc = tick_clock.global_clock
        n = len(gc)
        for i in range(n):
            if gc[i] <= 0:
                continue
            sub = [0] * n
            sub[i] = gc[i]
            nop = self.nc.sync.nop(nofuse=True)
            wait_clock.add_sem_waits(nop.ins, ScopedClock({None: VectorClock(sub)}))
        self.nc.sync.drain()
        self.nc.all_engine_barrier()
        popped = self.nc._tile_sem_poison_stack.pop()
        assert popped is self._sem_poison
        self.nc.clear_and_free_semaphores(list(self.sems.allocated().values()))
        self.nc.all_engine_barrier()


def build_program(S=2048, D=1024, H=16, DK=64, FF=4096, Q=512, EPS=1e-6):
    nd = D // P        # contraction chunks over D
    ns = S // P        # token tiles (full sequence)
    nq = Q // P        # token tiles (query slice)
    nf = FF // P       # chunks over FF
    DK1 = DK + 1       # head dim + denominator column
    HPG = P // DK      # heads per 128-partition group (2)
    n_sb = max(S // 512, 1)   # 512-wide column blocks over S
    SBW = S // n_sb
    n_dh = max(D // 512, 1)   # 512-wide column blocks over D
    DHW = D // n_dh
    assert H * DK == D and Q <= 512

    nc = bass.Bass("TRN2")

    xb = nc.declare_dram_parameter("xb", [S, D], F32, isOutput=False)
    xq = nc.declare_dram_parameter("xq", [Q, D], F32, isOutput=False)
    Wq = nc.declare_dram_parameter("Wq", [D, D], F32, isOutput=False)
    bq = nc.declare_dram_parameter("bq", [D], F32, isOutput=False)
    Wk = nc.declare_dram_parameter("Wk", [D, D], F32, isOutput=False)
    bk = nc.declare_dram_parameter("bk", [D], F32, isOutput=False)
    Wv = nc.declare_dram_parameter("Wv", [D, D], F32, isOutput=False)
    bv = nc.declare_dram_parameter("bv", [D], F32, isOutput=False)
    Wo = nc.declare_dram_parameter("Wo", [D, D], F32, isOutput=False)
    bo = nc.declare_dram_parameter("bo", [D], F32, isOutput=False)
    W1 = nc.declare_dram_parameter("W1", [D, FF], F32, isOutput=False)
    b1 = nc.declare_dram_parameter("b1", [FF], F32, isOutput=False)
    W2 = nc.declare_dram_parameter("W2", [FF, D], F32, isOutput=False)
    b2 = nc.declare_dram_parameter("b2", [D], F32, isOutput=False)
    ln1_g = nc.declare_dram_parameter("ln1_g", [D], F32, isOutput=False)
    ln1_b = nc.declare_dram_parameter("ln1_b", [D], F32, isOutput=False)
    ln2_g = nc.declare_dram_parameter("ln2_g", [D], F32, isOutput=False)
    ln2_b = nc.declare_dram_parameter("ln2_b", [D], F32, isOutput=False)
    out = nc.declare_dram_parameter("out", [Q, D], F32, isOutput=True)

    with _TC(nc) as tc:
      with tc.tile_pool(name="const", bufs=1) as cst:
        ident = cst.tile([P, P], BF16, tag="ident")
        make_identity(nc, ident)
        eps_t = cst.tile([P, 1], F32, tag="eps")
        nc.vector.memset(eps_t, EPS)
        ones_row = cst.tile([1, 512], BF16, tag="ones")
        nc.vector.memset(ones_row, 1.0)

        # per-partition layouts of per-feature vectors: d = c*P + p -> [p, c]
        ln1g_pm = cst.tile([P, nd], F32, tag="l1g")
        nc.sync.dma_start(out=ln1g_pm, in_=ln1_g[:].rearrange("(c p) -> p c", p=P))
        ln1b_pm = cst.tile([P, nd], F32, tag="l1b")
        nc.sync.dma_start(out=ln1b_pm, in_=ln1_b[:].rearrange("(c p) -> p c", p=P))
        b1_pm = cst.tile([P, nf], F32, tag="b1pm")
        nc.sync.dma_start(out=b1_pm, in_=b1[:].rearrange("(c p) -> p c", p=P))

        # LN2 affine broadcast over partitions (elementwise on xn2)
        g2b = cst.tile([P, D], F32, tag="g2b")
        nc.sync.dma_start(out=g2b, in_=_bcast(ln2_g[:], P))
        l2bb = cst.tile([P, D], F32, tag="l2bb")
        nc.sync.dma_start(out=l2bb, in_=_bcast(ln2_b[:], P))

        # plain bias rows (bf16) for PE outer-product folding
        bo_row = cst.tile([1, D], BF16, tag="bor")
        bo_f = cst.tile([1, D], F32, tag="bof")
        nc.sync.dma_start(out=bo_f, in_=_bcast(bo[:], 1))
        nc.vector.tensor_copy(bo_row, bo_f)
        b2_row = cst.tile([1, D], BF16, tag="b2r")
        b2_f = cst.tile([1, D], F32, tag="b2f")
        nc.sync.dma_start(out=b2_f, in_=_bcast(b2[:], 1))
        nc.vector.tensor_copy(b2_row, b2_f)

        with tc.tile_pool(name="bc", bufs=1) as bcp:      # O_T: phases B-C
          with tc.tile_pool(name="ab", bufs=1) as abp:    # KT/QT/Vt: phases A-B
            KT = [abp.tile([P, S], BF16, tag=f"kt{i}", name=f"KT{i}") for i in range(nd)]
            QT = [abp.tile([P, Q], BF16, tag=f"qt{i}", name=f"QT{i}") for i in range(nd)]
            Vt = [abp.tile([P, H, DK1], BF16, tag=f"vt{i}", name=f"Vt{i}") for i in range(ns)]
            for st in range(ns):
                nc.vector.memset(Vt[st][:, :, DK:DK1], 1.0)

            # ---------------- Phase A: LN1 + QKV projections ----------------
            with (
                tc.tile_pool(name="xio", bufs=2) as xio,
                tc.tile_pool(name="xn", bufs=3) as xnp,
                tc.tile_pool(name="xt", bufs=1) as xtp,
                tc.tile_pool(name="wf", bufs=2) as wfp,
                tc.tile_pool(name="wbf", bufs=12) as wbfp,
                tc.tile_pool(name="psA", bufs=3, space="PSUM") as psA,
                tc.tile_pool(name="psT", bufs=2, space="PSUM") as psT,
                tc.tile_pool(name="psB", bufs=2, space="PSUM") as psB,
                tc.tile_pool(name="stat", bufs=4) as stp,
            ):
                xn1T = [xtp.tile([P, S], BF16, tag=f"x1t{i}", name=f"xn1T{i}") for i in range(nd)]
                xqT = [xtp.tile([P, Q], BF16, tag=f"xqt{i}", name=f"xqT{i}") for i in range(nd)]

                def layernorm_tile(x_t, out_bf):
                    """token-major LN (stats+normalize, no affine), bf16 out."""
                    n_sub = max(D // 512, 1)
                    xr = x_t.rearrange("p (n f) -> p n f", n=n_sub)
                    stats = stp.tile([P, n_sub, 6], F32, tag="st", bufs=4, name="stats")
                    for su in range(n_sub):
                        nc.vector.bn_stats(out=stats[:, su, :], in_=xr[:, su, :])
                    mv = stp.tile([P, 2], F32, tag="mv", bufs=4, name="mv")
                    nc.vector.bn_aggr(out=mv, in_=stats)
                    stdt = stp.tile([P, 1], F32, tag="sd", bufs=4, name="stdt")
                    nc.scalar.activation(out=stdt, in_=mv[:, 1:2], func=AF.Sqrt, bias=eps_t)
                    rstd = stp.tile([P, 1], F32, tag="rs", bufs=4, name="rstd")
                    nc.vector.reciprocal(out=rstd, in_=stdt)
                    nc.vector.tensor_scalar(
                        out=out_bf, in0=x_t, scalar1=mv[:, 0:1], scalar2=rstd,
                        op0=OP.subtract, op1=OP.mult,
                    )

                def transpose_to(dst_tiles, src_bf, col0, par):
                    """src_bf [P, D] bf16 -> dst_tiles[k][:, col0:col0+P]."""
                    for k in range(nd):
                        pt = psT.tile([P, P], BF16, tag="pt", bufs=2, name="pt")
                        nc.tensor.transpose(pt, src_bf[:, k * P:(k + 1) * P], ident)
                        dst = dst_tiles[k][:, col0:col0 + P]
                        if (par + k) % 2:
                            nc.scalar.activation(out=dst, in_=pt, func=AF.Copy)
                        else:
                            nc.vector.tensor_copy(dst, pt)

                # LN1 over the full batch entry -> xn1T (feature-major)
                for st in range(ns):
                    x_t = xio.tile([P, D], F32, tag="xt", name="x_t")
                    nc.sync.dma_start(out=x_t, in_=xb[st * P:(st + 1) * P, :])
                    xn1 = xnp.tile([P, D], BF16, tag="xn1", name="xn1")
                    layernorm_tile(x_t, xn1)
                    transpose_to(xn1T, xn1, st * P, st)
                # LN1 over the query slice -> xqT
                for qt in range(nq):
                    x_t = xio.tile([P, D], F32, tag="xt", name="xq_t")
                    nc.sync.dma_start(out=x_t, in_=xq[qt * P:(qt + 1) * P, :])
                    xn1 = xnp.tile([P, D], BF16, tag="xn1", name="xqn1")
                    layernorm_tile(x_t, xn1)
                    transpose_to(xqT, xn1, qt * P, qt)

                # weights: f32 stage -> (bias matmul, g-scaled bf16 cast)
                def load_w_ln1(W_h, b_h, name):
                    wtiles = []
                    bps = [
                        psB.tile([1, DHW], F32, tag="bps", bufs=2, name=f"bp{name}{h}")
                        for h in range(n_dh)
                    ]
                    for k in range(nd):
                        wf_t = wfp.tile([P, D], F32, tag="wf", name=f"wf_{name}{k}")
                        nc.sync.dma_start(out=wf_t, in_=W_h[k * P:(k + 1) * P, :])
                        for h in range(n_dh):
                            nc.tensor.matmul(
                                bps[h], ln1b_pm[:, k:k + 1], wf_t[:, h * DHW:(h + 1) * DHW],
                                start=(k == 0), stop=(k == nd - 1),
                            )
                        wb = wbfp.tile([P, D], BF16, tag="wbf", name=f"wbf_{name}{k}")
                        nc.vector.tensor_scalar(
                            out=wb, in0=wf_t, scalar1=ln1g_pm[:, k:k + 1], scalar2=None,
                            op0=OP.mult,
                        )
                        wtiles.append(wb)
                    # bias_eff = ln1_b @ W + b  -> bf16 row [1, D]
                    be_f = stp.tile([1, D], F32, tag="bef", bufs=2, name=f"be_{name}")
                    nc.sync.dma_start(out=be_f, in_=_bcast(b_h[:], 1))
                    for h in range(n_dh):
                        nc.vector.tensor_tensor(
                            out=be_f[:, h * DHW:(h + 1) * DHW],
                            in0=be_f[:, h * DHW:(h + 1) * DHW], in1=bps[h], op=OP.add,
                        )
                    be = stp.tile([1, D], BF16, tag="beb", bufs=2, name=f"beb_{name}")
                    nc.vector.tensor_copy(be, be_f)
                    return wtiles, be

                # ---- Q^T = Wq'^T @ xqn^T ----
                Wq_bf, bqe = load_w_ln1(Wq, bq, "q")
                for cg in range(nd):
                    ps = psA.tile([P, Q], F32, tag="ps", bufs=3, name="ps_q")
                    nc.tensor.matmul(ps, bqe[:, cg * P:(cg + 1) * P], ones_row[:, :Q], start=True, stop=False)
                    for k in range(nd):
                        nc.tensor.matmul(
                            ps, Wq_bf[k][:, cg * P:(cg + 1) * P], xqT[k],
                            start=False, stop=(k == nd - 1),
                        )
                    if cg % 2:
                        nc.scalar.activation(out=QT[cg], in_=ps, func=AF.Copy)
                    else:
                        nc.vector.tensor_copy(QT[cg], ps)

                # ---- K^T = Wk'^T @ xn1^T ----
                Wk_bf, bke = load_w_ln1(Wk, bk, "k")
                for cg in range(nd):
                    for tg in range(n_sb):
                        ps = psA.tile([P, SBW], F32, tag="ps", bufs=3, name="ps_k")
                        nc.tensor.matmul(ps, bke[:, cg * P:(cg + 1) * P], ones_row[:, :SBW], start=True, stop=False)
                        for k in range(nd):
                            nc.tensor.matmul(
                                ps, Wk_bf[k][:, cg * P:(cg + 1) * P],
                                xn1T[k][:, tg * SBW:(tg + 1) * SBW],
                                start=False, stop=(k == nd - 1),
                            )
                        dst = KT[cg][:, tg * SBW:(tg + 1) * SBW]
                        if (cg + tg) % 2:
                            nc.scalar.activation(out=dst, in_=ps, func=AF.Copy)
                        else:
                            nc.vector.tensor_copy(dst, ps)

                # ---- V = xn1 @ Wv' (token-major, strided into [P,H,DK1]) ----
                Wv_bf, bve = load_w_ln1(Wv, bv, "v")
                for st in range(ns):
                    for hh in range(n_dh):
                        ps = psA.tile([P, DHW], F32, tag="ps", bufs=3, name="ps_v")
                        nc.tensor.matmul(ps, ones_row[:, :P], bve[:, hh * DHW:(hh + 1) * DHW], start=True, stop=False)
                        for k in range(nd):
                            nc.tensor.matmul(
                                ps, xn1T[k][:, st * P:(st + 1) * P],
                                Wv_bf[k][:, hh * DHW:(hh + 1) * DHW],
                                start=False, stop=(k == nd - 1),
                            )
                        hpb = DHW // DK  # heads per column block
                        dst = Vt[st][:, hh * hpb:(hh + 1) * hpb, 0:DK]
                        src = ps.rearrange("p (h d) -> p h d", d=DK)
                        if (st + hh) % 2:
                            nc.scalar.activation(out=dst, in_=src, func=AF.Copy)
                        else:
                            nc.vector.tensor_copy(dst, src)

            # ---------------- Phase B: attention ----------------
            O_T = [bcp.tile([P, Q], BF16, tag=f"ot{i}", name=f"O_T{i}") for i in range(nd)]
            with (
                tc.tile_pool(name="att", bufs=1) as att,
                tc.tile_pool(name="et", bufs=1) as etp,
                tc.tile_pool(name="psS", bufs=3, space="PSUM") as psS,
                tc.tile_pool(name="psO", bufs=3, space="PSUM") as psO,
                tc.tile_pool(name="psT2", bufs=2, space="PSUM") as psT2,
                tc.tile_pool(name="sc", bufs=8) as scp,
            ):
                O_sb = [att.tile([P, D], BF16, tag=f"os{i}", name=f"O_sb{i}") for i in range(nq)]
                for h in range(H):
                    cg, ro = h // HPG, (h % HPG) * DK
                    # E^T[kt] = exp(S^T / sqrt(DK)), S^T = K_h @ Q_h^T
                    e_tiles = []
                    for kt in range(ns):
                        ps = psS.tile([P, Q], F32, tag="pss", bufs=3, name="ps_s")
                        nc.tensor.matmul(
                            ps, KT[cg][ro:ro + DK, kt * P:(kt + 1) * P],
                            QT[cg][ro:ro + DK, :], start=True, stop=True,
                        )
                        et = etp.tile([P, Q], BF16, tag="et", bufs=20, name=f"et{h}_{kt}")
                        nc.scalar.activation(out=et, in_=ps, func=AF.Exp, scale=float(1.0 / np.sqrt(DK)))
                        e_tiles.append(et)
                    # O_h[qt] = (E^T)^T @ [V_h | 1] ; divide by ones-column
                    for qt in range(nq):
                        po = psO.tile([P, DK1], F32, tag="pso", bufs=3, name="ps_o")
                        for kt in range(ns):
                            nc.tensor.matmul(
                                po, e_tiles[kt][:, qt * P:(qt + 1) * P], Vt[kt][:, h, :],
                                start=(kt == 0), stop=(kt == ns - 1),
                            )
                        rec = scp.tile([P, 1], F32, tag="rec", bufs=8, name="rec")
                        nc.vector.reciprocal(out=rec, in_=po[:, DK:DK1])
                        nc.vector.tensor_scalar(
                            out=O_sb[qt][:, h * DK:(h + 1) * DK], in0=po[:, 0:DK],
                            scalar1=rec, scalar2=None, op0=OP.mult,
                        )
                # transpose O -> O_T (feature-major)
                for qt in range(nq):
                    for k in range(nd):
                        pt = psT2.tile([P, P], BF16, tag="pt2", bufs=2, name="pt2")
                        nc.tensor.transpose(pt, O_sb[qt][:, k * P:(k + 1) * P], ident)
                        dst = O_T[k][:, qt * P:(qt + 1) * P]
                        if (qt + k) % 2:
                            nc.scalar.activation(out=dst, in_=pt, func=AF.Copy)
                        else:
                            nc.vector.tensor_copy(dst, pt)

          # -------------- Phases C+D (x2 / xn2T live in both) --------------
          with tc.tile_pool(name="cd", bufs=1) as ccp:
            x2 = [ccp.tile([P, D], F32, tag=f"x2{i}", name=f"x2_{i}") for i in range(nq)]
            xn2T = [ccp.tile([P, Q], BF16, tag=f"x2t{i}", name=f"xn2T{i}") for i in range(nd)]

            # -------------- Phase C: O-proj + residual + LN2 --------------
            with (
                tc.tile_pool(name="wfo", bufs=2) as wfo,
                tc.tile_pool(name="wob", bufs=8) as wob,
                tc.tile_pool(name="xioc", bufs=2) as xioc,
                tc.tile_pool(name="psC", bufs=3, space="PSUM") as psC,
                tc.tile_pool(name="psT3", bufs=2, space="PSUM") as psT3,
                tc.tile_pool(name="statc", bufs=4) as stc,
            ):
                Wo_bf = []
                for k in range(nd):
                    wf_t = wfo.tile([P, D], F32, tag="wfo", name=f"wf_o{k}")
                    nc.sync.dma_start(out=wf_t, in_=Wo[k * P:(k + 1) * P, :])
                    wb = wob.tile([P, D], BF16, tag="wob", bufs=8, name=f"wo_bf{k}")
                    nc.gpsimd.tensor_copy(wb, wf_t)
                    Wo_bf.append(wb)
                for qt in range(nq):
                    xq_t = xioc.tile([P, D], F32, tag="xqc", name="xq_c")
                    nc.sync.dma_start(out=xq_t, in_=xq[qt * P:(qt + 1) * P, :])
                    for hh in range(n_dh):
                        ps = psC.tile([P, DHW], F32, tag="psc", bufs=3, name="ps_c")
                        nc.tensor.matmul(ps, ones_row[:, :P], bo_row[:, hh * DHW:(hh + 1) * DHW], start=True, stop=False)
                        for k in range(nd):
                            nc.tensor.matmul(
                                ps, O_T[k][:, qt * P:(qt + 1) * P],
                                Wo_bf[k][:, hh * DHW:(hh + 1) * DHW],
                                start=False, stop=(k == nd - 1),
                            )
                        nc.vector.tensor_tensor(
                            out=x2[qt][:, hh * DHW:(hh + 1) * DHW], in0=ps,
                            in1=xq_t[:, hh * DHW:(hh + 1) * DHW], op=OP.add,
                        )
                    # LN2 + affine (g2, b2 elementwise), then transpose
                    n_sub = max(D // 512, 1)
                    xr = x2[qt].rearrange("p (n f) -> p n f", n=n_sub)
                    stats = stc.tile([P, n_sub, 6], F32, tag="st2", bufs=4, name="stats2")
                    for su in range(n_sub):
                        nc.vector.bn_stats(out=stats[:, su, :], in_=xr[:, su, :])
                    mv = stc.tile([P, 2], F32, tag="mv2", bufs=4, name="mv2")
                    nc.vector.bn_aggr(out=mv, in_=stats)
                    stdt = stc.tile([P, 1], F32, tag="sd2", bufs=4, name="stdt2")
                    nc.scalar.activation(out=stdt, in_=mv[:, 1:2], func=AF.Sqrt, bias=eps_t)
                    rstd = stc.tile([P, 1], F32, tag="rs2", bufs=4, name="rstd2")
                    nc.vector.reciprocal(out=rstd, in_=stdt)
                    z_t = stc.tile([P, D], F32, tag="z2", bufs=2, name="z_t")
                    nc.vector.tensor_scalar(
                        out=z_t, in0=x2[qt], scalar1=mv[:, 0:1], scalar2=rstd,
                        op0=OP.subtract, op1=OP.mult,
                    )
                    nc.vector.tensor_tensor(out=z_t, in0=z_t, in1=g2b, op=OP.mult)
                    xn2 = stc.tile([P, D], BF16, tag="xn2", bufs=2, name="xn2")
                    nc.vector.tensor_tensor(out=xn2, in0=z_t, in1=l2bb, op=OP.add)
                    for k in range(nd):
                        pt = psT3.tile([P, P], BF16, tag="pt3", bufs=2, name="pt3")
                        nc.tensor.transpose(pt, xn2[:, k * P:(k + 1) * P], ident)
                        dst = xn2T[k][:, qt * P:(qt + 1) * P]
                        if (qt + k) % 2:
                            nc.scalar.activation(out=dst, in_=pt, func=AF.Copy)
                        else:
                            nc.vector.tensor_copy(dst, pt)

            # ---------------- Phase D: FFN + residual + out ----------------
            with (
                tc.tile_pool(name="wsg", bufs=2) as wsg,
                tc.tile_pool(name="wd", bufs=1) as wd,
                tc.tile_pool(name="h1", bufs=1) as h1p,
                tc.tile_pool(name="y2a", bufs=1) as y2p,
                tc.tile_pool(name="od", bufs=2) as odp,
                tc.tile_pool(name="psH", bufs=2, space="PSUM") as psH,
                tc.tile_pool(name="psY", bufs=2, space="PSUM") as psY,
            ):
                h1T = [h1p.tile([P, Q], BF16, tag=f"h1{i}", name=f"h1T{i}") for i in range(nf)]
                y2a = [y2p.tile([P, D], F32, tag=f"ya{i}", name=f"y2a{i}") for i in range(nq)]
                nfh = max(nf // 2, 1)        # ff chunks per half
                FFW = nfh * P                # ff columns per half
                n_w1stage = max(FFW // 1024, 1)
                W1W = FFW // n_w1stage

                def load_w1_half(half):
                    tiles = []
                    for k in range(nd):
                        parts = []
                        for j in range(n_w1stage):
                            c0 = half * FFW + j * W1W
                            wst = wsg.tile([P, W1W], F32, tag="wsg", name=f"w1s{half}_{k}_{j}")
                            nc.sync.dma_start(out=wst, in_=W1[k * P:(k + 1) * P, c0:c0 + W1W])
                            wbt = wd.tile([P, W1W], BF16, tag="w1", bufs=nd * n_w1stage + 2,
                                          name=f"w1b{half}_{k}_{j}")
                            nc.gpsimd.tensor_copy(wbt, wst)
                            parts.append(wbt)
                        tiles.append(parts)
                    return tiles

                def load_w2_half(half):
                    tiles = []
                    for fc in range(half * nfh, (half + 1) * nfh):
                        wst = wsg.tile([P, D], F32, tag="wsg", name=f"w2s{fc}")
                        nc.sync.dma_start(out=wst, in_=W2[fc * P:(fc + 1) * P, :])
                        wbt = wd.tile([P, D], BF16, tag="w2", bufs=nfh + 2, name=f"w2b{fc}")
                        nc.gpsimd.tensor_copy(wbt, wst)
                        tiles.append(wbt)
                    return tiles

                def h1_half(w1_tiles, half):
                    for fc in range(half * nfh, (half + 1) * nfh):
                        ps = psH.tile([P, Q], F32, tag="psh", bufs=2, name="ps_h")
                        lc = fc - half * nfh
                        j, jo = divmod(lc * P, W1W)
                        for k in range(nd):
                            nc.tensor.matmul(
                                ps, w1_tiles[k][j][:, jo:jo + P], xn2T[k],
                                start=(k == 0), stop=(k == nd - 1),
                            )
                        nc.scalar.activation(
                            out=h1T[fc], in_=ps, func=AF.Relu, bias=b1_pm[:, fc:fc + 1],
                        )

                w1a = load_w1_half(0)
                h1_half(w1a, 0)
                w1b = load_w1_half(1)
                w2a = load_w2_half(0)
                # y2a = h1[:, :FFW] @ W2[:FFW] (drained to SBUF)
                for qt in range(nq):
                    for hh in range(n_dh):
                        ps = psY.tile([P, DHW], F32, tag="psy", bufs=2, name="ps_ya")
                        for i, fc in enumerate(range(0, nfh)):
                            nc.tensor.matmul(
                                ps, h1T[fc][:, qt * P:(qt + 1) * P],
                                w2a[i][:, hh * DHW:(hh + 1) * DHW],
                                start=(i == 0), stop=(i == nfh - 1),
                            )
                        dst = y2a[qt][:, hh * DHW:(hh + 1) * DHW]
                        if (qt + hh) % 2:
                            nc.scalar.activation(out=dst, in_=ps, func=AF.Copy)
                        else:
                            nc.vector.tensor_copy(dst, ps)
                h1_half(w1b, 1)
                w2b = load_w2_half(1)
                for qt in range(nq):
                    o_t = odp.tile([P, D], F32, tag="od", name="o_t")
                    for hh in range(n_dh):
                        ps = psY.tile([P, DHW], F32, tag="psy", bufs=2, name="ps_yb")
                        nc.tensor.matmul(ps, ones_row[:, :P], b2_row[:, hh * DHW:(hh + 1) * DHW], start=True, stop=False)
                        for i, fc in enumerate(range(nfh, nf)):
                            nc.tensor.matmul(
                                ps, h1T[fc][:, qt * P:(qt + 1) * P],
                                w2b[i][:, hh * DHW:(hh + 1) * DHW],
                                start=False, stop=(i == nfh - 1),
                            )
                        sl = slice(hh * DHW, (hh + 1) * DHW)
                        nc.vector.tensor_tensor(out=o_t[:, sl], in0=ps, in1=y2a[qt][:, sl], op=OP.add)
                        nc.vector.tensor_tensor(out=o_t[:, sl], in0=o_t[:, sl], in1=x2[qt][:, sl], op=OP.add)
                    nc.sync.dma_start(out=out[qt * P:(qt + 1) * P, :], in_=o_t)

    return nc


_MAXW = 1  # max sync waits walrus accepts per instruction here


def _split_waits_json(raw: bytes) -> bytes:
    """Split multi-wait instructions: excess sync waits move onto
    preceding single-wait EventSemaphore instructions on the same
    engine (the engine stalls there, gating everything it issues
    afterwards -- semantically identical, codegen-legal)."""
    import json as _json

    d = _json.loads(raw)
    ctr = 0
    for f in d.get("functions", []):
        for bb in f.get("blocks", []):
            insts = bb.get("instructions", [])
            out = []
            for ins in insts:
                si = ins.get("sync_info")
                waits = si.get("on_wait") if si else None
                if waits and len(waits) > _MAXW:
                    for w in waits[:-_MAXW]:
                        ctr += 1
                        out.append({
                            "debug": ins.get("debug", 0),
                            "engine": ins["engine"],
                            "ins": [],
                            "outs": [],
                            "name": f"wsplit-{ctr}",
                            "opcode": "EventSemaphore",
                            "sync_info": {"on_update": [], "on_wait": [w]},
                        })
                    si["on_wait"] = waits[-_MAXW:]
                out.append(ins)
            bb["instructions"] = out
    return _json.dumps(d).encode()


def _patch_serialization(nc):
    orig = nc.to_json_bytes

    def patched():
        return _split_waits_json(orig())

    nc.to_json_bytes = patched
    return nc


_CACHED = {}


def _get_nc():
    if "nc" not in _CACHED:
        _CACHED["nc"] = _patch_serialization(build_program())
    return _CACHED["nc"]


def make_in_maps(inputs):
    x = np.ascontiguousarray(np.asarray(inputs["x"], dtype=np.float32))
    B, S, D = x.shape
    QW = B * S // 8
    shared = {
        k: np.ascontiguousarray(np.asarray(inputs[k], dtype=np.float32))
        for k in ("Wq", "bq", "Wk", "bk", "Wv", "bv", "Wo", "bo",
                  "W1", "b1", "W2", "b2", "ln1_g", "ln1_b", "ln2_g", "ln2_b")
    }
    gpb = 8 // B  # cores per batch entry
    in_maps = []
    for c in range(8):
        b, g = c // gpb, c % gpb
        m = dict(shared)
        m["xb"] = x[b]
        m["xq"] = np.ascontiguousarray(x[b][g * QW:(g + 1) * QW])
        in_maps.append(m)
    return in_maps


def kernel(**inputs) -> np.ndarray:
    from concourse.bass_utils import run_bass_kernel_spmd

    x = np.asarray(inputs["x"])
    B, S, D = x.shape
    QW = B * S // 8
    gpb = 8 // B
    nc = _get_nc()
    res = run_bass_kernel_spmd(nc, make_in_maps(inputs), core_ids=list(range(8)))
    out = np.empty((B, S, D), dtype=np.float32)
    for c in range(8):
        b, g = c // gpb, c % gpb
        out[b, g * QW:(g + 1) * QW] = res.results[c]["out"]
    return out
